# revision 3
# baseline (speedup 1.0000x reference)
"""AttentivePooling Trainium2 kernel, v6 (bf16 single-pass full-width chunks).

Reference semantics (h_all: [T, B, D] f32, xin unused):
    h_last = h_all[-1]
    a[b, t] = <h_all[t, b, :], h_last[b, :]> / sqrt(D)
    r = relu(a); w = r / (sum_t r + 1e-9)
    out[b, d] = sum_t w[b, t] * h_all[t, b, d]

Single pass: since relu(s*a) = s*relu(a) exactly, accumulate unnormalized
P_b = sum_t relu(s*a) h and Z_b = sum_t relu(s*a); out = P/(Z+1e-9) at the
end.  Data-parallel over B across 8 cores, 8 batches/core.

v6 moves the whole on-chip pipeline to bf16 (h is cast f32->bf16 by the
SWDGE DMA engines in flight): halves SBUF write traffic and engine read
traffic, unlocks DVE's 2x tensor_tensor mode, and drops all the f32r
games.  Per [128, 4096] chunk (16 KiB contiguous DRAM lines):
  - DVE multiplies cols [0:3072) against the h_last broadcast, GPSIMD
    [3072:4096); both write bf16
  - score reduce: ACT activation-accum for batch segments 0-3, one DVE 3D
    tensor_reduce for segments 4-7
  - ACT relu (scale=1/sqrt(D)) -> weights in the persistent wall tile
  - 8 bf16 matmuls accumulate P into 8 single-row PSUM banks
h_last broadcast: one 16 KiB row DMA + PE outer-product (no DMA fabric
cost, no GPSIMD ucode-library stall).  Z: one end matmul over the wall
(reusing bank 0's slot) + a tiny DVE reduce.
"""

import numpy as np
from contextlib import ExitStack

import concourse.bass as bass
import concourse.tile as tile
from concourse import bacc, mybir
from concourse.bass_utils import run_bass_kernel_spmd

T, B, D = 2048, 64, 512
NCORES = 8
BPC = B // NCORES
P = 128
TC = T // P
BW = BPC * D
SCALE = float(1.0 / np.sqrt(np.float32(D)))
HBUFS = 12  # h-chunk pipeline depth (bf16 chunks are 8 KiB/partition)
PRELOAD = 5  # chunks dispatched before the loop (SWDGE dispatch is ~1us each)
TMPBUFS = 4
DVE_COLS = 3072  # multiply split: DVE [0:3072), GPSIMD [3072:4096)
ACT_SEGS = 4  # score-reduce split: segs 0-3 on ACT, 4-7 as one DVE 3D reduce

_nc_cache = None


def _build():
    global _nc_cache
    if _nc_cache is not None:
        return _nc_cache
    nc = bacc.Bacc("TRN2", debug=False, target_bir_lowering=False, num_devices=NCORES)
    h = nc.dram_tensor("h", [T, BPC, D], mybir.dt.float32, kind="ExternalInput")
    out = nc.dram_tensor("out", [BPC, D], mybir.dt.float32, kind="ExternalOutput")
    f32 = mybir.dt.float32
    bf16 = mybir.dt.bfloat16
    hflat = h.ap().rearrange("t b d -> t (b d)")  # [2048, 4096]
    out_ap = out.ap()

    with tile.TileContext(nc) as tc:
        with ExitStack() as ctx:
            hpool = ctx.enter_context(tc.tile_pool(name="h", bufs=HBUFS))
            hlpool = ctx.enter_context(tc.tile_pool(name="hl", bufs=1))
            tmpp = ctx.enter_context(tc.tile_pool(name="tmp", bufs=TMPBUFS))
            scrp = ctx.enter_context(tc.tile_pool(name="scr", bufs=4))
            smallp = ctx.enter_context(tc.tile_pool(name="small", bufs=1))
            constp = ctx.enter_context(tc.tile_pool(name="const", bufs=1))
            psoutp = ctx.enter_context(tc.tile_pool(name="pso", bufs=1, space="PSUM"))

            eps_tile = constp.tile([1, 1], f32, name="eps")
            nc.vector.memset(eps_tile[:], 1e-9)
            # bf16 ones for the Z matmul (lhsT) and the broadcast (lhsT row)
            ones_f = constp.tile([P, 1], f32, name="ones_f")
            nc.vector.memset(ones_f[:], 1.0)
            ones_col = constp.tile([P, 1], bf16, name="ones_col")
            nc.scalar.copy(ones_col[:], ones_f[:])
            ones_row_f = constp.tile([1, P], f32, name="ones_row_f")
            nc.vector.memset(ones_row_f[:], 1.0)
            ones_row = constp.tile([1, P], bf16, name="ones_row")
            nc.scalar.copy(ones_row[:], ones_row_f[:])

            # h_last: one 16 KiB row DMA (cast to bf16 by SWDGE), then PE
            # outer-product broadcast to 128 partitions through two reused
            # PSUM bank slots; copies write the bf16 broadcast tile.
            hlrow = hlpool.tile([1, BW], bf16, name="hlrow")
            nc.gpsimd.dma_start(hlrow[:], hflat[T - 1 : T, :])
            hl = hlpool.tile([P, BW], bf16, name="hl")
            for s in range(BPC):
                bcb = psoutp.tile([P, D], f32, tag=f"pb{6 + (s % 2)}", name=f"bc{s}")
                nc.tensor.matmul(
                    bcb[:],
                    ones_row[:],
                    hlrow[0:1, s * D : (s + 1) * D],
                    start=True,
                    stop=True,
                )
                if s % 2:
                    nc.scalar.copy(hl[:, s * D : (s + 1) * D], bcb[:])
                else:
                    nc.vector.tensor_copy(hl[:, s * D : (s + 1) * D], bcb[:])

            def load_chunk(c):
                t = hpool.tile([P, BW], bf16, tag="hc", name="h_sb")
                nc.gpsimd.dma_start(t[:], hflat[c * P : (c + 1) * P, :])
                return t

            # relu'd weights for every chunk live here; also the Z source
            wall = constp.tile([P, TC, BPC], bf16, name="wall")

            # one [1, D] f32 PSUM bank per batch (PE outputs must start at
            # partition 0)
            pbanks = [psoutp.tile([1, D], f32, name=f"pb{k}") for k in range(BPC)]

            h_tiles = {c: load_chunk(c) for c in range(PRELOAD)}
            issued = PRELOAD

            for c in range(TC):
                for _ in range(2):
                    if issued < min(TC, c + HBUFS + 1):
                        h_tiles[issued] = load_chunk(issued)
                        issued += 1

                hc = h_tiles.pop(c)

                tmp = tmpp.tile([P, BW], bf16, tag="tmp")
                nc.vector.tensor_tensor(
                    tmp[:, 0:DVE_COLS],
                    hc[:, 0:DVE_COLS],
                    hl[:, 0:DVE_COLS],
                    mybir.AluOpType.mult,
                )
                nc.gpsimd.tensor_tensor(
                    tmp[:, DVE_COLS:BW],
                    hc[:, DVE_COLS:BW],
                    hl[:, DVE_COLS:BW],
                    mybir.AluOpType.mult,
                )

                scr = scrp.tile([P, BPC], f32, tag="scr")
                for b in range(ACT_SEGS):
                    seg = tmp[:, b * D : (b + 1) * D]
                    nc.scalar.activation(
                        seg,
                        seg,
                        mybir.ActivationFunctionType.Copy,
                        accum_out=scr[:, b : b + 1],
                    )
                # segs 4-7: fold the 2048 columns in half with a bf16 2x
                # TT add, then 1x-reduce only 1024 columns
                pairs = tmp[:, ACT_SEGS * D : BPC * D].rearrange(
                    "p (s two d) -> p s two d", two=2, d=D // 2
                )
                tmp2 = scrp.tile([P, BPC - ACT_SEGS, D // 2], bf16, tag="t2")
                nc.vector.tensor_tensor(
                    tmp2[:],
                    pairs[:, :, 0, :],
                    pairs[:, :, 1, :],
                    mybir.AluOpType.add,
                )
                nc.vector.tensor_reduce(
                    scr[:, ACT_SEGS:BPC],
                    tmp2[:],
                    mybir.AxisListType.X,
                    mybir.AluOpType.add,
                )

                # weights = relu(scores * 1/sqrt(D)); exact vs reference
                w = wall[:, c, :]
                nc.scalar.activation(
                    w, scr[:], mybir.ActivationFunctionType.Relu, scale=SCALE
                )

                for b in range(BPC):
                    nc.tensor.matmul(
                        pbanks[b][:],
                        w[:, b : b + 1],
                        hc[:, b * D : (b + 1) * D],
                        start=(c == 0),
                        stop=(c == TC - 1),
                    )

            # tail: free bank pb0, reduce Z there, normalize, store
            p0c = smallp.tile([1, D], f32, tag="p0c")
            nc.scalar.copy(p0c[:], pbanks[0][:])
            pzall = psoutp.tile([1, TC * BPC], f32, tag="pb0")
            nc.tensor.matmul(
                pzall[:],
                ones_col[:],
                wall[:].rearrange("p c b -> p (c b)"),
                start=True,
                stop=True,
            )
            pz = smallp.tile([1, BPC], f32, tag="pzs")
            nc.vector.tensor_reduce(
                pz[:],
                pzall[:].rearrange("o (c b) -> o b c", b=BPC),
                mybir.AxisListType.X,
                mybir.AluOpType.add,
            )
            zeps = smallp.tile([1, BPC], f32, tag="zeps")
            nc.scalar.activation(
                zeps[:],
                pz[:],
                mybir.ActivationFunctionType.Identity,
                bias=eps_tile[0:1, 0:1],
            )
            zrec = smallp.tile([1, BPC], f32, tag="zrec")
            nc.vector.reciprocal(zrec[:], zeps[:])
            res = smallp.tile([1, BW], f32, tag="res")
            nc.vector.tensor_scalar_mul(res[0:1, 0:D], p0c[:], zrec[0:1, 0:1])
            for b in range(1, BPC):
                if b % 2:
                    nc.scalar.mul(
                        res[0:1, b * D : (b + 1) * D],
                        pbanks[b][:],
                        zrec[0:1, b : b + 1],
                    )
                else:
                    nc.vector.tensor_scalar_mul(
                        res[0:1, b * D : (b + 1) * D],
                        pbanks[b][:],
                        zrec[0:1, b : b + 1],
                    )
            nc.sync.dma_start(out_ap.rearrange("(o b) d -> o (b d)", o=1), res[:])

    nc.finalize()
    _nc_cache = nc
    return nc


def _run(h_all: np.ndarray, trace: bool = False):
    nc = _build()
    h_all = np.ascontiguousarray(np.asarray(h_all), dtype=np.float32)
    assert h_all.shape == (T, B, D)
    in_maps = [
        {"h": np.ascontiguousarray(h_all[:, c * BPC : (c + 1) * BPC, :])}
        for c in range(NCORES)
    ]
    r = run_bass_kernel_spmd(nc, in_maps, list(range(NCORES)), trace=trace)
    out = np.concatenate([r.results[c]["out"] for c in range(NCORES)], axis=0)
    return out, r


def kernel(h_all: np.ndarray, xin: np.ndarray | None = None) -> np.ndarray:
    out, _ = _run(h_all)
    return out


# revision 4
# speedup vs baseline: 1.1667x; 1.1667x over previous
"""AttentivePooling Trainium2 kernel, v6 (bf16 single-pass full-width chunks).

Reference semantics (h_all: [T, B, D] f32, xin unused):
    h_last = h_all[-1]
    a[b, t] = <h_all[t, b, :], h_last[b, :]> / sqrt(D)
    r = relu(a); w = r / (sum_t r + 1e-9)
    out[b, d] = sum_t w[b, t] * h_all[t, b, d]

Single pass: since relu(s*a) = s*relu(a) exactly, accumulate unnormalized
P_b = sum_t relu(s*a) h and Z_b = sum_t relu(s*a); out = P/(Z+1e-9) at the
end.  Data-parallel over B across 8 cores, 8 batches/core.

v6 moves the whole on-chip pipeline to bf16 (h is cast f32->bf16 by the
SWDGE DMA engines in flight): halves SBUF write traffic and engine read
traffic, unlocks DVE's 2x tensor_tensor mode, and drops all the f32r
games.  Per [128, 4096] chunk (16 KiB contiguous DRAM lines):
  - DVE multiplies cols [0:3072) against the h_last broadcast, GPSIMD
    [3072:4096); both write bf16
  - score reduce: ACT activation-accum for batch segments 0-3, one DVE 3D
    tensor_reduce for segments 4-7
  - ACT relu (scale=1/sqrt(D)) -> weights in the persistent wall tile
  - 8 bf16 matmuls accumulate P into 8 single-row PSUM banks
h_last broadcast: one 16 KiB row DMA + PE outer-product (no DMA fabric
cost, no GPSIMD ucode-library stall).  Z: one end matmul over the wall
(reusing bank 0's slot) + a tiny DVE reduce.
"""

import numpy as np
from contextlib import ExitStack

import concourse.bass as bass
import concourse.tile as tile
from concourse import bacc, mybir
from concourse.bass_utils import run_bass_kernel_spmd

T, B, D = 2048, 64, 512
NCORES = 8
BPC = B // NCORES
P = 128
TC = T // P
BW = BPC * D
SCALE = float(1.0 / np.sqrt(np.float32(D)))
HBUFS = 12  # h-chunk pipeline depth (bf16 chunks are 8 KiB/partition)
PRELOAD = 5  # chunks dispatched before the loop (SWDGE dispatch is ~1us each)
TMPBUFS = 4
DVE_COLS = 3072  # multiply split: DVE [0:3072), GPSIMD [3072:4096)
ACT_SEGS = 4  # score-reduce split: segs 0-3 on ACT, 4-7 as one DVE 3D reduce

_nc_cache = None


def _build():
    global _nc_cache
    if _nc_cache is not None:
        return _nc_cache
    nc = bacc.Bacc("TRN2", debug=False, target_bir_lowering=False, num_devices=NCORES)
    h = nc.dram_tensor("h", [T, BPC, D], mybir.dt.float32, kind="ExternalInput")
    out = nc.dram_tensor("out", [BPC, D], mybir.dt.float32, kind="ExternalOutput")
    f32 = mybir.dt.float32
    bf16 = mybir.dt.bfloat16
    hflat = h.ap().rearrange("t b d -> t (b d)")  # [2048, 4096]
    out_ap = out.ap()

    with tile.TileContext(nc) as tc:
        with ExitStack() as ctx:
            hpool = ctx.enter_context(tc.tile_pool(name="h", bufs=HBUFS))
            hlpool = ctx.enter_context(tc.tile_pool(name="hl", bufs=1))
            tmpp = ctx.enter_context(tc.tile_pool(name="tmp", bufs=TMPBUFS))
            scrp = ctx.enter_context(tc.tile_pool(name="scr", bufs=4))
            smallp = ctx.enter_context(tc.tile_pool(name="small", bufs=1))
            constp = ctx.enter_context(tc.tile_pool(name="const", bufs=1))
            psoutp = ctx.enter_context(tc.tile_pool(name="pso", bufs=1, space="PSUM"))

            eps_tile = constp.tile([1, 1], f32, name="eps")
            nc.vector.memset(eps_tile[:], 1e-9)
            # bf16 ones for the Z matmul (lhsT) and the broadcast (lhsT row)
            ones_f = constp.tile([P, 1], f32, name="ones_f")
            nc.vector.memset(ones_f[:], 1.0)
            ones_col = constp.tile([P, 1], bf16, name="ones_col")
            nc.scalar.copy(ones_col[:], ones_f[:])
            ones_row_f = constp.tile([1, P], f32, name="ones_row_f")
            nc.vector.memset(ones_row_f[:], 1.0)
            ones_row = constp.tile([1, P], bf16, name="ones_row")
            nc.scalar.copy(ones_row[:], ones_row_f[:])

            # h_last: one 16 KiB row DMA (cast to bf16 by SWDGE), then PE
            # outer-product broadcast to 128 partitions through two reused
            # PSUM bank slots; copies write the bf16 broadcast tile.
            hlrow = hlpool.tile([1, BW], bf16, name="hlrow")
            nc.gpsimd.dma_start(hlrow[:], hflat[T - 1 : T, :])
            hl = hlpool.tile([P, BW], bf16, name="hl")
            for s in range(BPC):
                bcb = psoutp.tile([P, D], f32, tag=f"pb{6 + (s % 2)}", name=f"bc{s}")
                nc.tensor.matmul(
                    bcb[:],
                    ones_row[:],
                    hlrow[0:1, s * D : (s + 1) * D],
                    start=True,
                    stop=True,
                )
                if s % 2:
                    nc.scalar.copy(hl[:, s * D : (s + 1) * D], bcb[:])
                else:
                    nc.vector.tensor_copy(hl[:, s * D : (s + 1) * D], bcb[:])

            def load_chunk(c):
                t = hpool.tile([P, BW], bf16, tag="hc", name="h_sb")
                nc.gpsimd.dma_start(t[:], hflat[c * P : (c + 1) * P, :])
                return t

            # relu'd weights for every chunk live here; also the Z source
            wall = constp.tile([P, TC, BPC], bf16, name="wall")

            # one [1, D] f32 PSUM bank per batch (PE outputs must start at
            # partition 0)
            pbanks = [psoutp.tile([1, D], f32, name=f"pb{k}") for k in range(BPC)]

            h_tiles = {c: load_chunk(c) for c in range(PRELOAD)}
            issued = PRELOAD

            for c in range(TC):
                for _ in range(2):
                    if issued < min(TC, c + HBUFS + 1):
                        h_tiles[issued] = load_chunk(issued)
                        issued += 1

                hc = h_tiles.pop(c)

                tmp = tmpp.tile([P, BW], bf16, tag="tmp")
                nc.vector.tensor_tensor(
                    tmp[:, 0:DVE_COLS],
                    hc[:, 0:DVE_COLS],
                    hl[:, 0:DVE_COLS],
                    mybir.AluOpType.mult,
                )
                nc.gpsimd.tensor_tensor(
                    tmp[:, DVE_COLS:BW],
                    hc[:, DVE_COLS:BW],
                    hl[:, DVE_COLS:BW],
                    mybir.AluOpType.mult,
                )

                scr = scrp.tile([P, BPC], f32, tag="scr")
                for b in range(ACT_SEGS):
                    seg = tmp[:, b * D : (b + 1) * D]
                    nc.scalar.activation(
                        seg,
                        seg,
                        mybir.ActivationFunctionType.Copy,
                        accum_out=scr[:, b : b + 1],
                    )
                # segs 4-7: fold the 2048 columns in half with a bf16 2x
                # TT add, then 1x-reduce only 1024 columns
                pairs = tmp[:, ACT_SEGS * D : BPC * D].rearrange(
                    "p (s two d) -> p s two d", two=2, d=D // 2
                )
                tmp2 = scrp.tile([P, BPC - ACT_SEGS, D // 2], bf16, tag="t2")
                nc.vector.tensor_tensor(
                    tmp2[:],
                    pairs[:, :, 0, :],
                    pairs[:, :, 1, :],
                    mybir.AluOpType.add,
                )
                # folded seg 4 reduces on ACT (cheap at 256 cols); 5-7 on DVE
                nc.scalar.activation(
                    tmp2[:, 0, :],
                    tmp2[:, 0, :],
                    mybir.ActivationFunctionType.Copy,
                    accum_out=scr[:, ACT_SEGS : ACT_SEGS + 1],
                )
                nc.vector.tensor_reduce(
                    scr[:, ACT_SEGS + 1 : BPC],
                    tmp2[:, 1:, :],
                    mybir.AxisListType.X,
                    mybir.AluOpType.add,
                )

                # weights = relu(scores * 1/sqrt(D)); exact vs reference
                w = wall[:, c, :]
                nc.scalar.activation(
                    w, scr[:], mybir.ActivationFunctionType.Relu, scale=SCALE
                )

                for b in range(BPC):
                    nc.tensor.matmul(
                        pbanks[b][:],
                        w[:, b : b + 1],
                        hc[:, b * D : (b + 1) * D],
                        start=(c == 0),
                        stop=(c == TC - 1),
                    )

            # tail: free bank pb0, reduce Z there, normalize, store
            p0c = smallp.tile([1, D], f32, tag="p0c")
            nc.scalar.copy(p0c[:], pbanks[0][:])
            pzall = psoutp.tile([1, TC * BPC], f32, tag="pb0")
            nc.tensor.matmul(
                pzall[:],
                ones_col[:],
                wall[:].rearrange("p c b -> p (c b)"),
                start=True,
                stop=True,
            )
            pz = smallp.tile([1, BPC], f32, tag="pzs")
            nc.vector.tensor_reduce(
                pz[:],
                pzall[:].rearrange("o (c b) -> o b c", b=BPC),
                mybir.AxisListType.X,
                mybir.AluOpType.add,
            )
            zeps = smallp.tile([1, BPC], f32, tag="zeps")
            nc.scalar.activation(
                zeps[:],
                pz[:],
                mybir.ActivationFunctionType.Identity,
                bias=eps_tile[0:1, 0:1],
            )
            zrec = smallp.tile([1, BPC], f32, tag="zrec")
            nc.vector.reciprocal(zrec[:], zeps[:])
            res = smallp.tile([1, BW], f32, tag="res")
            nc.vector.tensor_scalar_mul(res[0:1, 0:D], p0c[:], zrec[0:1, 0:1])
            for b in range(1, BPC):
                if b % 2:
                    nc.scalar.mul(
                        res[0:1, b * D : (b + 1) * D],
                        pbanks[b][:],
                        zrec[0:1, b : b + 1],
                    )
                else:
                    nc.vector.tensor_scalar_mul(
                        res[0:1, b * D : (b + 1) * D],
                        pbanks[b][:],
                        zrec[0:1, b : b + 1],
                    )
            nc.sync.dma_start(out_ap.rearrange("(o b) d -> o (b d)", o=1), res[:])

    nc.finalize()
    _nc_cache = nc
    return nc


def _run(h_all: np.ndarray, trace: bool = False):
    nc = _build()
    h_all = np.ascontiguousarray(np.asarray(h_all), dtype=np.float32)
    assert h_all.shape == (T, B, D)
    in_maps = [
        {"h": np.ascontiguousarray(h_all[:, c * BPC : (c + 1) * BPC, :])}
        for c in range(NCORES)
    ]
    r = run_bass_kernel_spmd(nc, in_maps, list(range(NCORES)), trace=trace)
    out = np.concatenate([r.results[c]["out"] for c in range(NCORES)], axis=0)
    return out, r


def kernel(h_all: np.ndarray, xin: np.ndarray | None = None) -> np.ndarray:
    out, _ = _run(h_all)
    return out
